# revision 1
# baseline (speedup 1.0000x reference)
"""HANConv Trainium2 kernel (8 NeuronCores, SPMD, full-I/O contract).

Strategy
--------
Destination-sharded, fully core-independent:
  * Each core owns 1/8 of destination nodes for BOTH relations
    (writes: author->paper, written: paper->author).
  * Edges are sorted by destination window (128 dst rows) on host. Per
    window, source rows are gathered from a bf16 copy of the raw source
    features via gpsimd.dma_gather (int16 indices => lo/hi table split),
    and segment-summed with one-hot matmuls accumulating in PSUM.
  * Aggregating RAW features (M = A_norm @ x) lets the relation transform
    and the semantic-score transform both become dense matmuls from M with
    host-folded weights (W_rel, W_rel @ W_sem), so no cross-core exchange
    of transformed features is ever needed.
  * Self path is computed from host-transposed x slices with folded
    weights (W_self, W_self @ W_sem) so no on-chip transpose is needed.
  * 2-candidate semantic softmax == sigmoid of score difference.
"""

import sys

sys.path.insert(0, "/opt/trn_rl_repo")

import numpy as np
import ml_dtypes

import concourse.bacc as bacc
import concourse.mybir as mybir
import concourse.tile as tile
from concourse.bass_utils import run_bass_kernel_spmd

P = 128
N = 50000
D = 256
HALF = 32768  # int16 gather index limit
NCORES = 8
NW_TOTAL = (N + P - 1) // P            # 391 destination windows
NWIN = (NW_TOTAL + NCORES - 1) // NCORES  # 49 windows per core
NW_ALLOC = NWIN * NCORES               # 392 (incl. 1 phantom window)
NPAD = NWIN * P                        # 6272 output rows per core

BF16 = ml_dtypes.bfloat16
F32 = np.float32

# (mps, tps, dps, sb, gbuf, oh) pool bufs
POOL_CFG = (2, 1, 1, 4, 4, 4)


# ---------------------------------------------------------------- host prep
def _prep_relation(row, col):
    """Sort edges by (dst window, src half); pad each group to 128 multiples.

    Returns idx16 [16, NW_ALLOC, 8*call], colf [P, NW_ALLOC, call],
    recip [P, NW_ALLOC], c_lo, c_hi.
    """
    E = row.shape[0]
    key = (col // P) * 2 + (row >= HALF)
    order = np.argsort(key, kind="stable")
    ks = key[order]
    rs = row[order].astype(np.int64)
    cs = col[order].astype(np.int64)

    counts = np.bincount(key, minlength=NW_TOTAL * 2)
    lo_cnt = counts[0::2]
    hi_cnt = counts[1::2]
    c_lo = max(1, int(-(-lo_cnt.max() // P)))
    c_hi = max(1, int(-(-hi_cnt.max() // P)))
    call = c_lo + c_hi

    grp_start = np.zeros(NW_TOTAL * 2 + 1, dtype=np.int64)
    np.cumsum(counts, out=grp_start[1:])
    rank = np.arange(E, dtype=np.int64) - grp_start[ks]
    w_of = ks // 2
    hi_of = ks % 2

    idx_flat = np.zeros(NW_ALLOC * call * P, dtype=np.int16)
    col_flat = np.full(NW_ALLOC * call * P, -1.0, dtype=F32)
    pos = w_of * (call * P) + hi_of * (c_lo * P) + rank
    idx_flat[pos] = (rs - HALF * hi_of).astype(np.int16)
    col_flat[pos] = (cs - w_of * P).astype(F32)

    idx_all = idx_flat.reshape(NW_ALLOC, call * P)
    # wrap for dma_gather: idx i of a gather block -> [i % 16, i // 16]
    lo_wr = idx_all[:, : c_lo * P].reshape(NW_ALLOC, c_lo * 8, 16).transpose(2, 0, 1)
    hi_wr = idx_all[:, c_lo * P:].reshape(NW_ALLOC, c_hi * 8, 16).transpose(2, 0, 1)
    idx16 = np.concatenate([lo_wr, hi_wr], axis=2)  # [16, NW_ALLOC, 8*call]

    colf = col_flat.reshape(NW_ALLOC, call, P).transpose(2, 0, 1)  # [P, NW, call]

    deg = np.bincount(col, minlength=NW_ALLOC * P).astype(F32)[: NW_ALLOC * P]
    recip = (1.0 / np.maximum(deg, 1.0)).reshape(NW_ALLOC, P).T  # [P, NW]
    return idx16, colf, recip, c_lo, c_hi


def _host_prep(inp):
    pr = {}
    pr["wr"] = _prep_relation(np.asarray(inp["row_writes"]), np.asarray(inp["col_writes"]))
    pr["wn"] = _prep_relation(np.asarray(inp["row_written"]), np.asarray(inp["col_written"]))

    xa = np.asarray(inp["x_author"], dtype=F32)
    xp = np.asarray(inp["x_paper"], dtype=F32)
    pr["xba"] = xa.astype(BF16)
    pr["xbp"] = xp.astype(BF16)

    # per-core transposed x slices (for the self path of the dst shard)
    xta, xtp = [], []
    for c in range(NCORES):
        r0, r1 = c * NPAD, min(N, (c + 1) * NPAD)
        sa = np.zeros((D, NPAD), dtype=BF16)
        sp = np.zeros((D, NPAD), dtype=BF16)
        sa[:, : r1 - r0] = xa[r0:r1].T
        sp[:, : r1 - r0] = xp[r0:r1].T
        xta.append(sa)
        xtp.append(sp)
    pr["xta"], pr["xtp"] = xta, xtp

    W_sem = np.asarray(inp["W_sem"], dtype=F32)
    b_sem = np.asarray(inp["b_sem"], dtype=F32)
    w_score = np.asarray(inp["w_score"], dtype=F32)

    def w(name):
        return np.asarray(inp[name], dtype=F32)

    pr["w_self_a"] = w("W_self_author").astype(BF16)
    pr["w_self_p"] = w("W_self_paper").astype(BF16)
    pr["wf_self_a"] = (w("W_self_author") @ W_sem).astype(BF16)
    pr["wf_self_p"] = (w("W_self_paper") @ W_sem).astype(BF16)
    pr["w_rel_wr"] = w("W_rel_writes").astype(BF16)
    pr["w_rel_wn"] = w("W_rel_written").astype(BF16)
    pr["wf_rel_wr"] = (w("W_rel_writes") @ W_sem).astype(BF16)
    pr["wf_rel_wn"] = (w("W_rel_written") @ W_sem).astype(BF16)

    rep = lambda v: np.tile(v.astype(F32), (P, 1))
    pr["b_self_a_rep"] = rep(w("b_self_author"))
    pr["b_self_p_rep"] = rep(w("b_self_paper"))
    pr["bf_self_a_rep"] = rep(w("b_self_author") @ W_sem + b_sem)
    pr["bf_self_p_rep"] = rep(w("b_self_paper") @ W_sem + b_sem)
    pr["bsem_rep"] = rep(b_sem)
    pr["w_rep"] = rep(w_score)

    pr["iota"] = np.tile(np.arange(P, dtype=F32), (P, 1)).astype(BF16)
    pr["ident"] = np.eye(P, dtype=F32).astype(BF16)
    return pr


# ---------------------------------------------------------------- program
def build_program(nwin, c_lo_wr, c_hi_wr, c_lo_wn, c_hi_wn):
    f32 = mybir.dt.float32
    bf16 = mybir.dt.bfloat16
    i16 = mybir.dt.int16
    AF = mybir.ActivationFunctionType
    OP = mybir.AluOpType

    call_wr = c_lo_wr + c_hi_wr
    call_wn = c_lo_wn + c_hi_wn
    npad = nwin * P

    nc = bacc.Bacc("TRN2", target_bir_lowering=False, debug=False)

    _mb, _tb, _db, _sb, _gb, _ob = POOL_CFG

    xba = nc.dram_tensor("xba", [N, D], bf16, kind="ExternalInput")
    xbp = nc.dram_tensor("xbp", [N, D], bf16, kind="ExternalInput")
    xta = nc.dram_tensor("xta", [D, npad], bf16, kind="ExternalInput")
    xtp = nc.dram_tensor("xtp", [D, npad], bf16, kind="ExternalInput")

    wnames = ["w_self_a", "wf_self_a", "w_self_p", "wf_self_p",
              "w_rel_wr", "wf_rel_wr", "w_rel_wn", "wf_rel_wn"]
    wdram = {n: nc.dram_tensor(n, [D, D], bf16, kind="ExternalInput") for n in wnames}
    bnames = ["b_self_a_rep", "bf_self_a_rep", "b_self_p_rep", "bf_self_p_rep",
              "bsem_rep", "w_rep"]
    bdram = {n: nc.dram_tensor(n, [P, D], f32, kind="ExternalInput") for n in bnames}
    iota_d = nc.dram_tensor("iota", [P, P], bf16, kind="ExternalInput")
    ident_d = nc.dram_tensor("ident", [P, P], bf16, kind="ExternalInput")

    idx_wr_d = nc.dram_tensor("idx_wr", [P, nwin * 8 * call_wr], i16, kind="ExternalInput")
    idx_wn_d = nc.dram_tensor("idx_wn", [P, nwin * 8 * call_wn], i16, kind="ExternalInput")
    colf_wr_d = nc.dram_tensor("colf_wr", [P, nwin * call_wr], bf16, kind="ExternalInput")
    colf_wn_d = nc.dram_tensor("colf_wn", [P, nwin * call_wn], bf16, kind="ExternalInput")
    recip_wr_d = nc.dram_tensor("recip_wr", [P, nwin], f32, kind="ExternalInput")
    recip_wn_d = nc.dram_tensor("recip_wn", [P, nwin], f32, kind="ExternalInput")

    oa = nc.dram_tensor("oa", [npad, D], f32, kind="ExternalOutput")
    op_ = nc.dram_tensor("op", [npad, D], f32, kind="ExternalOutput")

    with tile.TileContext(nc) as tc:
        with tc.tile_pool(name="const", bufs=1) as cpool, \
             tc.tile_pool(name="gbuf", bufs=_gb) as gpool, \
             tc.tile_pool(name="oh", bufs=_ob) as ohpool, \
             tc.tile_pool(name="sb", bufs=_sb) as sbpool, \
             tc.tile_pool(name="mps", bufs=_mb, space="PSUM") as mpool, \
             tc.tile_pool(name="tps", bufs=_tb, space="PSUM") as tpool, \
             tc.tile_pool(name="dps", bufs=_db, space="PSUM") as dpool:

            def load(dram, shape, dtype, tag):
                t = cpool.tile(shape, dtype, tag=tag)
                nc.sync.dma_start(t[:], dram)
                return t

            iota_t = load(iota_d[:], [P, P], bf16, "c_iota")
            ident_t = load(ident_d[:], [P, P], bf16, "c_ident")
            wt = {n: (load(wdram[n][0:P, :], [P, D], bf16, f"c_{n}0"),
                      load(wdram[n][P:D, :], [P, D], bf16, f"c_{n}1")) for n in wnames}
            bt = {n: load(bdram[n][:], [P, D], f32, f"c_{n}") for n in bnames}
            xta_t = (load(xta[0:P, :], [P, npad], bf16, "c_xta0"),
                     load(xta[P:D, :], [P, npad], bf16, "c_xta1"))
            xtp_t = (load(xtp[0:P, :], [P, npad], bf16, "c_xtp0"),
                     load(xtp[P:D, :], [P, npad], bf16, "c_xtp1"))
            idx_wr_t = load(idx_wr_d[:], [P, nwin * 8 * call_wr], i16, "c_idxwr")
            idx_wn_t = load(idx_wn_d[:], [P, nwin * 8 * call_wn], i16, "c_idxwn")
            colf_wr_t = load(colf_wr_d[:], [P, nwin * call_wr], bf16, "c_colfwr")
            colf_wn_t = load(colf_wn_d[:], [P, nwin * call_wn], bf16, "c_colfwn")
            recip_wr_t = load(recip_wr_d[:], [P, nwin], f32, "c_recipwr")
            recip_wn_t = load(recip_wn_d[:], [P, nwin], f32, "c_recipwn")

            rels = [
                dict(tag="wr", table=xba, idx=idx_wr_t, colf=colf_wr_t,
                     recip=recip_wr_t, c_lo=c_lo_wr, c_hi=c_hi_wr,
                     xt=xtp_t, w_self=wt["w_self_p"], wf_self=wt["wf_self_p"],
                     w_rel=wt["w_rel_wr"], wf_rel=wt["wf_rel_wr"],
                     b_self=bt["b_self_p_rep"], bf_self=bt["bf_self_p_rep"],
                     out=op_),
                dict(tag="wn", table=xbp, idx=idx_wn_t, colf=colf_wn_t,
                     recip=recip_wn_t, c_lo=c_lo_wn, c_hi=c_hi_wn,
                     xt=xta_t, w_self=wt["w_self_a"], wf_self=wt["wf_self_a"],
                     w_rel=wt["w_rel_wn"], wf_rel=wt["wf_rel_wn"],
                     b_self=bt["b_self_a_rep"], bf_self=bt["bf_self_a_rep"],
                     out=oa),
            ]

            def emit_window(w, r):
                c_lo, c_hi = r["c_lo"], r["c_hi"]
                call = c_lo + c_hi
                ic0 = w * 8 * call

                g_lo = gpool.tile([P, c_lo, D], bf16, tag="glo")
                nc.gpsimd.dma_gather(
                    g_lo[:], r["table"][:], r["idx"][:, ic0: ic0 + 8 * c_lo],
                    c_lo * P, c_lo * P, D, single_packet=False)
                g_hi = gpool.tile([P, c_hi, D], bf16, tag="ghi")
                nc.gpsimd.dma_gather(
                    g_hi[:], r["table"][HALF:, :],
                    r["idx"][:, ic0 + 8 * c_lo: ic0 + 8 * call],
                    c_hi * P, c_hi * P, D, single_packet=False)

                oh = ohpool.tile([P, call, P], bf16, tag="oh")
                nc.vector.tensor_tensor(
                    out=oh[:],
                    in0=r["colf"][:, w * call: (w + 1) * call, None].to_broadcast([P, call, P]),
                    in1=iota_t[:, None, :].to_broadcast([P, call, P]),
                    op=OP.is_equal)

                m_ps = mpool.tile([P, D], f32, tag="m")
                for k in range(call):
                    rhs = g_lo[:, k, :] if k < c_lo else g_hi[:, k - c_lo, :]
                    nc.tensor.matmul(out=m_ps[:], lhsT=oh[:, k, :], rhs=rhs,
                                     start=(k == 0), stop=(k == call - 1))

                m_sb = sbpool.tile([P, D], bf16, tag="m_sb")
                nc.vector.tensor_tensor(
                    out=m_sb[:], in0=m_ps[:],
                    in1=r["recip"][:, w: w + 1].to_broadcast([P, D]), op=OP.mult)

                mt = []
                for h2 in range(2):
                    t_ps = tpool.tile([P, P], bf16, tag="t")
                    nc.tensor.transpose(out=t_ps[:], in_=m_sb[:, h2 * P: (h2 + 1) * P],
                                        identity=ident_t[:])
                    mt_sb = sbpool.tile([P, P], bf16, tag=f"mt{h2}")
                    nc.vector.tensor_copy(out=mt_sb[:], in_=t_ps[:])
                    mt.append(mt_sb)

                def dense(lhsT0, lhsT1, wpair, ptag, pool=dpool):
                    ps = pool.tile([P, D], f32, tag=ptag)
                    nc.tensor.matmul(out=ps[:], lhsT=lhsT0, rhs=wpair[0][:],
                                     start=True, stop=False)
                    nc.tensor.matmul(out=ps[:], lhsT=lhsT1, rhs=wpair[1][:],
                                     start=False, stop=True)
                    return ps

                agg_ps = dense(mt[0][:], mt[1][:], r["w_rel"], "agg")
                sarg_ps = dense(mt[0][:], mt[1][:], r["wf_rel"], "sarg")
                xsl0 = r["xt"][0][:, w * P: (w + 1) * P]
                xsl1 = r["xt"][1][:, w * P: (w + 1) * P]
                h_ps = dense(xsl0, xsl1, r["w_self"], "h")
                sh_ps = dense(xsl0, xsl1, r["wf_self"], "sh")

                def score(ps, brep, stag):
                    targ = sbpool.tile([P, D], f32, tag=f"targ{stag}")
                    nc.vector.tensor_add(out=targ[:], in0=ps[:], in1=brep[:])
                    ttan = sbpool.tile([P, D], f32, tag=f"ttan{stag}")
                    nc.scalar.activation(out=ttan[:], in_=targ[:], func=AF.Tanh)
                    scr = sbpool.tile([P, D], f32, tag=f"scr{stag}")
                    nc.vector.tensor_mul(out=scr[:], in0=ttan[:], in1=bt["w_rep"][:])
                    s = sbpool.tile([P, 1], f32, tag=f"s{stag}")
                    nc.vector.tensor_reduce(out=s[:], in_=scr[:],
                                            axis=mybir.AxisListType.X,
                                            op=OP.add)
                    return s

                s_agg = score(sarg_ps, bt["bsem_rep"], "a")
                s_h = score(sh_ps, r["bf_self"], "h")

                h_sb = sbpool.tile([P, D], f32, tag="h_sb")
                nc.vector.tensor_add(out=h_sb[:], in0=h_ps[:], in1=r["b_self"][:])

                dsc = sbpool.tile([P, 1], f32, tag="dsc")
                nc.vector.tensor_sub(out=dsc[:], in0=s_h[:], in1=s_agg[:])
                a0 = sbpool.tile([P, 1], f32, tag="a0")
                nc.scalar.activation(out=a0[:], in_=dsc[:], func=AF.Sigmoid)

                diff = sbpool.tile([P, D], f32, tag="diff")
                nc.vector.tensor_sub(out=diff[:], in0=h_sb[:], in1=agg_ps[:])
                wd = sbpool.tile([P, D], f32, tag="wd")
                nc.vector.tensor_tensor(out=wd[:], in0=diff[:],
                                        in1=a0[:, 0:1].to_broadcast([P, D]),
                                        op=OP.mult)
                outt = sbpool.tile([P, D], f32, tag="outt")
                nc.vector.tensor_add(out=outt[:], in0=wd[:], in1=agg_ps[:])
                nc.sync.dma_start(r["out"][w * P: (w + 1) * P, :], outt[:])

            for w in range(nwin):
                for r in rels:
                    emit_window(w, r)

    nc.compile()
    return nc


# ---------------------------------------------------------------- driver
_PROG_CACHE = {}


def _get_program(key):
    if key not in _PROG_CACHE:
        _PROG_CACHE[key] = build_program(*key)
    return _PROG_CACHE[key]


def _make_in_maps(pr):
    shared = dict(
        xba=pr["xba"], xbp=pr["xbp"],
        iota=pr["iota"], ident=pr["ident"],
        bsem_rep=pr["bsem_rep"], w_rep=pr["w_rep"],
        b_self_a_rep=pr["b_self_a_rep"], b_self_p_rep=pr["b_self_p_rep"],
        bf_self_a_rep=pr["bf_self_a_rep"], bf_self_p_rep=pr["bf_self_p_rep"],
        w_self_a=pr["w_self_a"], w_self_p=pr["w_self_p"],
        wf_self_a=pr["wf_self_a"], wf_self_p=pr["wf_self_p"],
        w_rel_wr=pr["w_rel_wr"], w_rel_wn=pr["w_rel_wn"],
        wf_rel_wr=pr["wf_rel_wr"], wf_rel_wn=pr["wf_rel_wn"],
    )
    idx_wr, colf_wr, recip_wr, _, _ = pr["wr"]
    idx_wn, colf_wn, recip_wn, _, _ = pr["wn"]
    in_maps = []
    for c in range(NCORES):
        w0, w1 = c * NWIN, (c + 1) * NWIN
        m = dict(shared)
        m["xta"] = pr["xta"][c]
        m["xtp"] = pr["xtp"][c]
        m["idx_wr"] = np.ascontiguousarray(
            np.tile(idx_wr[:, w0:w1].reshape(16, -1), (8, 1)))
        m["idx_wn"] = np.ascontiguousarray(
            np.tile(idx_wn[:, w0:w1].reshape(16, -1), (8, 1)))
        m["colf_wr"] = np.ascontiguousarray(colf_wr[:, w0:w1].reshape(P, -1)).astype(BF16)
        m["colf_wn"] = np.ascontiguousarray(colf_wn[:, w0:w1].reshape(P, -1)).astype(BF16)
        m["recip_wr"] = np.ascontiguousarray(recip_wr[:, w0:w1])
        m["recip_wn"] = np.ascontiguousarray(recip_wn[:, w0:w1])
        in_maps.append(m)
    return in_maps


def run(trace=False, tmpdir=None, **inputs):
    pr = _host_prep(inputs)
    _, _, _, c_lo_wr, c_hi_wr = pr["wr"]
    _, _, _, c_lo_wn, c_hi_wn = pr["wn"]
    nc = _get_program((NWIN, c_lo_wr, c_hi_wr, c_lo_wn, c_hi_wn))
    in_maps = _make_in_maps(pr)
    res = run_bass_kernel_spmd(nc, in_maps, list(range(NCORES)),
                               trace=trace, tmpdir=tmpdir)
    oa = np.empty((N, D), dtype=F32)
    op = np.empty((N, D), dtype=F32)
    for c in range(NCORES):
        r0, r1 = c * NPAD, min(N, (c + 1) * NPAD)
        oa[r0:r1] = res.results[c]["oa"][: r1 - r0]
        op[r0:r1] = res.results[c]["op"][: r1 - r0]
    return (oa, op), res


def kernel(**inputs):
    (oa, op), _ = run(trace=False, **inputs)
    return (oa, op)



# revision 17
# speedup vs baseline: 1.3315x; 1.3315x over previous
"""HANConv Trainium2 kernel (8 NeuronCores, SPMD, full-I/O contract).

Strategy (v2)
-------------
Destination-sharded, fully core-independent:
  * Each core owns 1/8 of destination nodes for BOTH relations
    (writes: author->paper, written: paper->author).
  * Edges are sorted by (dst window, src half, src) on host. Per window,
    source rows are gathered as fp8(e4m3) 256B rows via gpsimd.dma_gather,
    round-robin over 4 SWDGE queues (4x the single-queue descriptor
    throughput; the gather is descriptor-bound, so fp8 also halves bytes),
    and segment-summed with fp8 one-hot matmuls accumulating in f32 PSUM.
  * Aggregating RAW features (M = A @ x, then per-dst 1/deg scale on the
    scalar engine) lets every later transform be a dense matmul from M with
    host-folded weights, so no cross-core exchange is ever needed.
  * 2-candidate semantic softmax is rewritten tanh-only:
        out = p + tanh(0.5*(s_h - s_agg)) * q
        p = 0.5*(h + agg),  q = 0.5*(h - agg)
    with the 0.5 factors folded into the weights on host. The scalar
    engine therefore never switches activation tables.
  * Scores use one fused DVE tensor_tensor_reduce:
        dsc = 0.5 * sum(w_score * (tanh(z_h) - tanh(z_agg)))
  * Self path computed from host-transposed x slices with folded weights.
  * Outputs written bf16 and upcast to f32 on host.
"""

import sys

sys.path.insert(0, "/opt/trn_rl_repo")

import numpy as np
import ml_dtypes

import concourse.bacc as bacc
import concourse.mybir as mybir
import concourse.tile as tile
from concourse.bass_utils import run_bass_kernel_spmd

P = 128
N = 50000
D = 256
HALF = 32768  # int16 gather index limit
NCORES = 8
NW_TOTAL = (N + P - 1) // P            # 391 destination windows
NWIN = (NW_TOTAL + NCORES - 1) // NCORES  # 49 windows per core
NW_ALLOC = NWIN * NCORES               # 392 (incl. 1 phantom window)
NPAD = NWIN * P                        # 6272 output rows per core

BF16 = ml_dtypes.bfloat16
FP8 = ml_dtypes.float8_e4m3
F32 = np.float32

USE_FP8 = True
NQ = 4


# ---------------------------------------------------------------- host prep
def _prep_relation(row, col):
    """Sort edges by (dst window, src half, src); pad groups to 128.

    Returns idx16 [16, NW_ALLOC, 8*call], colf [P, NW_ALLOC, call],
    recip [P, NW_ALLOC], c_lo, c_hi.
    """
    E = row.shape[0]
    key = (col // P) * 2 + (row >= HALF)
    order = np.lexsort((row, key))
    ks = key[order]
    rs = row[order].astype(np.int64)
    cs = col[order].astype(np.int64)

    counts = np.bincount(key, minlength=NW_TOTAL * 2)
    lo_cnt = counts[0::2]
    hi_cnt = counts[1::2]
    c_lo = max(1, int(-(-lo_cnt.max() // P)))
    c_hi = max(1, int(-(-hi_cnt.max() // P)))
    call = c_lo + c_hi

    grp_start = np.zeros(NW_TOTAL * 2 + 1, dtype=np.int64)
    np.cumsum(counts, out=grp_start[1:])
    rank = np.arange(E, dtype=np.int64) - grp_start[ks]
    w_of = ks // 2
    hi_of = ks % 2

    idx_flat = np.zeros(NW_ALLOC * call * P, dtype=np.int16)
    col_flat = np.full(NW_ALLOC * call * P, -1.0, dtype=F32)
    pos = w_of * (call * P) + hi_of * (c_lo * P) + rank
    idx_flat[pos] = (rs - HALF * hi_of).astype(np.int16)
    col_flat[pos] = (cs - w_of * P).astype(F32)

    idx_all = idx_flat.reshape(NW_ALLOC, call * P)
    # wrap for dma_gather: idx i of a gather block -> [i % 16, i // 16]
    lo_wr = idx_all[:, : c_lo * P].reshape(NW_ALLOC, c_lo * 8, 16).transpose(2, 0, 1)
    hi_wr = idx_all[:, c_lo * P:].reshape(NW_ALLOC, c_hi * 8, 16).transpose(2, 0, 1)
    idx16 = np.concatenate([lo_wr, hi_wr], axis=2)  # [16, NW_ALLOC, 8*call]

    colf = col_flat.reshape(NW_ALLOC, call, P).transpose(2, 0, 1)  # [P, NW, call]

    deg = np.bincount(col, minlength=NW_ALLOC * P).astype(F32)[: NW_ALLOC * P]
    recip = (1.0 / np.maximum(deg, 1.0)).reshape(NW_ALLOC, P).T  # [P, NW]
    return idx16, colf, recip, c_lo, c_hi


def _host_prep(inp):
    pr = {}
    pr["wr"] = _prep_relation(np.asarray(inp["row_writes"]), np.asarray(inp["col_writes"]))
    pr["wn"] = _prep_relation(np.asarray(inp["row_written"]), np.asarray(inp["col_written"]))

    xa = np.asarray(inp["x_author"], dtype=F32)
    xp = np.asarray(inp["x_paper"], dtype=F32)
    if USE_FP8:
        # fp8 gather tables, viewed as bf16 [N, 128] for the byte-moving gather
        pr["xa8"] = xa.astype(FP8).view(np.uint16).view(BF16)
        pr["xp8"] = xp.astype(FP8).view(np.uint16).view(BF16)
    else:
        pr["xa8"] = xa.astype(BF16)
        pr["xp8"] = xp.astype(BF16)

    # per-core transposed x slices (for the self path of the dst shard)
    xta, xtp = [], []
    for c in range(NCORES):
        r0, r1 = c * NPAD, min(N, (c + 1) * NPAD)
        sa = np.zeros((D, NPAD), dtype=BF16)
        sp = np.zeros((D, NPAD), dtype=BF16)
        sa[:, : r1 - r0] = xa[r0:r1].T
        sp[:, : r1 - r0] = xp[r0:r1].T
        xta.append(sa)
        xtp.append(sp)
    pr["xta"], pr["xtp"] = xta, xtp

    W_sem = np.asarray(inp["W_sem"], dtype=F32)
    b_sem = np.asarray(inp["b_sem"], dtype=F32)
    w_score = np.asarray(inp["w_score"], dtype=F32)

    def w(name):
        return np.asarray(inp[name], dtype=F32)

    # folded weights per relation: (dst self weight, rel weight)
    for tag, wself, bself, wrel in (
        ("wr", w("W_self_paper"), w("b_self_paper"), w("W_rel_writes")),
        ("wn", w("W_self_author"), w("b_self_author"), w("W_rel_written")),
    ):
        pr[f"wp_self_{tag}"] = (0.5 * wself).astype(BF16)
        pr[f"wp_rel_{tag}"] = (0.5 * wrel).astype(BF16)
        pr[f"wq_rel_{tag}"] = (-0.5 * wrel).astype(BF16)
        pr[f"wf_self_{tag}"] = (wself @ W_sem).astype(BF16)
        pr[f"wf_rel_{tag}"] = (wrel @ W_sem).astype(BF16)
        # bias rows: [1, 3*D] = (0.5*b_self | b_self@W_sem + b_sem | b_sem)
        pr[f"brows_{tag}"] = np.concatenate([
            0.5 * bself, bself @ W_sem + b_sem, b_sem,
        ]).reshape(1, 3 * D).astype(BF16)

    # pre-scaled by the 0.5 from sigmoid(x) = 0.5*(1+tanh(x/2))
    pr["wrep"] = np.tile(0.5 * w_score, (P, 1)).astype(F32)
    pr["iota"] = np.tile(np.arange(P, dtype=F32), (P, 1)).astype(BF16)
    pr["ident"] = np.eye(P, dtype=F32).astype(BF16)
    pr["ones"] = np.ones((1, P), dtype=BF16)
    return pr


# ---------------------------------------------------------------- program
def build_program(nwin, c_lo_wr, c_hi_wr, c_lo_wn, c_hi_wn, scale=1,
                  nq=4, use_fp8=True):
    f32 = mybir.dt.float32
    bf16 = mybir.dt.bfloat16
    f8 = mybir.dt.float8e4 if use_fp8 else mybir.dt.bfloat16
    i16 = mybir.dt.int16
    AF = mybir.ActivationFunctionType
    OP = mybir.AluOpType

    call_wr = c_lo_wr + c_hi_wr
    call_wn = c_lo_wn + c_hi_wn
    npad = nwin * P

    nc = bacc.Bacc("TRN2", target_bir_lowering=False, debug=False,
                   num_swdge_queues=nq)

    TW = P if use_fp8 else D
    xa8 = nc.dram_tensor("xa8", [N, TW], bf16, kind="ExternalInput")
    xp8 = nc.dram_tensor("xp8", [N, TW], bf16, kind="ExternalInput")
    xta = nc.dram_tensor("xta", [D, npad], bf16, kind="ExternalInput")
    xtp = nc.dram_tensor("xtp", [D, npad], bf16, kind="ExternalInput")

    wnames = []
    for tag in ("wr", "wn"):
        wnames += [f"wp_self_{tag}", f"wp_rel_{tag}", f"wq_rel_{tag}",
                   f"wf_self_{tag}", f"wf_rel_{tag}"]
    wdram = {n: nc.dram_tensor(n, [D, D], bf16, kind="ExternalInput") for n in wnames}
    brow_d = {tag: nc.dram_tensor(f"brows_{tag}", [1, 3 * D], bf16,
                                  kind="ExternalInput") for tag in ("wr", "wn")}
    wrep_d = nc.dram_tensor("wrep", [P, D], f32, kind="ExternalInput")
    iota_d = nc.dram_tensor("iota", [P, P], bf16, kind="ExternalInput")
    ident_d = nc.dram_tensor("ident", [P, P], bf16, kind="ExternalInput")
    ones_d = nc.dram_tensor("ones", [1, P], bf16, kind="ExternalInput")

    idx_wr_d = nc.dram_tensor("idx_wr", [P, nwin * 8 * call_wr], i16, kind="ExternalInput")
    idx_wn_d = nc.dram_tensor("idx_wn", [P, nwin * 8 * call_wn], i16, kind="ExternalInput")
    colf_wr_d = nc.dram_tensor("colf_wr", [P, nwin * call_wr], bf16, kind="ExternalInput")
    colf_wn_d = nc.dram_tensor("colf_wn", [P, nwin * call_wn], bf16, kind="ExternalInput")
    recip_wr_d = nc.dram_tensor("recip_wr", [P, nwin], f32, kind="ExternalInput")
    recip_wn_d = nc.dram_tensor("recip_wn", [P, nwin], f32, kind="ExternalInput")

    oa = nc.dram_tensor("oa", [npad, D], bf16, kind="ExternalOutput")
    op_ = nc.dram_tensor("op", [npad, D], bf16, kind="ExternalOutput")

    with tile.TileContext(nc) as tc:
        with tc.tile_pool(name="const", bufs=1) as cpool, \
             tc.tile_pool(name="gbuf", bufs=3) as gpool, \
             tc.tile_pool(name="oh", bufs=3) as ohpool, \
             tc.tile_pool(name="sb", bufs=3) as sbpool, \
             tc.tile_pool(name="mps", bufs=2, space="PSUM") as mpool, \
             tc.tile_pool(name="tps", bufs=1, space="PSUM") as tpool, \
             tc.tile_pool(name="dps", bufs=1, space="PSUM") as dpool:

            def load(dram, shape, dtype, tag):
                t = cpool.tile(shape, dtype, tag=tag)
                nc.sync.dma_start(t[:], dram)
                return t

            iota_t = load(iota_d[:], [P, P], bf16, "c_iota")
            ident_t = load(ident_d[:], [P, P], bf16, "c_ident")
            ones_t = load(ones_d[:], [1, P], bf16, "c_ones")
            wrep_t = load(wrep_d[:], [P, D], f32, "c_wrep")
            wt = {n: (load(wdram[n][0:P, :], [P, D], bf16, f"c_{n}0"),
                      load(wdram[n][P:D, :], [P, D], bf16, f"c_{n}1")) for n in wnames}
            brow = {tag: load(brow_d[tag][:], [1, 3 * D], bf16, f"c_br{tag}")
                    for tag in ("wr", "wn")}
            xta_t = (load(xta[0:P, :], [P, npad], bf16, "c_xta0"),
                     load(xta[P:D, :], [P, npad], bf16, "c_xta1"))
            xtp_t = (load(xtp[0:P, :], [P, npad], bf16, "c_xtp0"),
                     load(xtp[P:D, :], [P, npad], bf16, "c_xtp1"))
            idx_wr_t = load(idx_wr_d[:], [P, nwin * 8 * call_wr], i16, "c_idxwr")
            idx_wn_t = load(idx_wn_d[:], [P, nwin * 8 * call_wn], i16, "c_idxwn")
            colf_wr_t = load(colf_wr_d[:], [P, nwin * call_wr], bf16, "c_colfwr")
            colf_wn_t = load(colf_wn_d[:], [P, nwin * call_wn], bf16, "c_colfwn")
            recip_wr_t = load(recip_wr_d[:], [P, nwin], f32, "c_recipwr")
            recip_wn_t = load(recip_wn_d[:], [P, nwin], f32, "c_recipwn")

            rels = [
                dict(tag="wr", table=xa8, idx=idx_wr_t, colf=colf_wr_t,
                     recip=recip_wr_t, c_lo=c_lo_wr, c_hi=c_hi_wr,
                     xt=xtp_t, q0=0, out=op_),
                dict(tag="wn", table=xp8, idx=idx_wn_t, colf=colf_wn_t,
                     recip=recip_wn_t, c_lo=c_lo_wn, c_hi=c_hi_wn,
                     xt=xta_t, q0=2, out=oa),
            ]
            for r in rels:
                tag = r["tag"]
                r["wp_self"] = wt[f"wp_self_{tag}"]
                r["wp_rel"] = wt[f"wp_rel_{tag}"]
                r["wq_rel"] = wt[f"wq_rel_{tag}"]
                r["wf_self"] = wt[f"wf_self_{tag}"]
                r["wf_rel"] = wt[f"wf_rel_{tag}"]
                r["brow"] = brow[tag]

            def emit_window(w, r):
                tag = r["tag"]
                c_lo, c_hi = r["c_lo"], r["c_hi"]
                call = c_lo + c_hi
                ic0 = w * 8 * call

                g_lo = gpool.tile([P, c_lo, D], f8, tag=f"glo{tag}")
                nc.gpsimd.dma_gather(
                    g_lo.bitcast(bf16) if use_fp8 else g_lo[:], r["table"][:],
                    r["idx"][:, ic0: ic0 + 8 * c_lo],
                    c_lo * P, c_lo * P, TW, single_packet=False,
                    queue_num=r["q0"] % nq)
                g_hi = gpool.tile([P, c_hi, D], f8, tag=f"ghi{tag}")
                nc.gpsimd.dma_gather(
                    g_hi.bitcast(bf16) if use_fp8 else g_hi[:], r["table"][HALF:, :],
                    r["idx"][:, ic0 + 8 * c_lo: ic0 + 8 * call],
                    c_hi * P, c_hi * P, TW, single_packet=False,
                    queue_num=(r["q0"] + 1) % nq)

                oh = ohpool.tile([P, call, P], f8, tag=f"oh{tag}")
                nc.vector.tensor_tensor(
                    out=oh[:],
                    in0=r["colf"][:, w * call: (w + 1) * call, None].to_broadcast([P, call, P]),
                    in1=iota_t[:, None, :].to_broadcast([P, call, P]),
                    op=OP.is_equal)

                m_ps = mpool.tile([P, D], f32, tag="m")
                for k in range(call):
                    rhs = g_lo[:, k, :] if k < c_lo else g_hi[:, k - c_lo, :]
                    nc.tensor.matmul(out=m_ps[:], lhsT=oh[:, k, :], rhs=rhs,
                                     start=(k == 0), stop=(k == call - 1))

                # deg-normalize on the scalar engine (per-dst 1/deg scale)
                m_sb = sbpool.tile([P, D], bf16, tag="m_sb")
                nc.scalar.activation(out=m_sb[:], in_=m_ps[:], func=AF.Copy,
                                     scale=r["recip"][:, w: w + 1])

                mt = []
                for h2 in range(2):
                    t_ps = tpool.tile([P, P], bf16, tag=f"t{h2}")
                    nc.tensor.transpose(out=t_ps[:],
                                        in_=m_sb[:, h2 * P: (h2 + 1) * P],
                                        identity=ident_t[:])
                    mt_sb = sbpool.tile([P, P], bf16, tag=f"mt{h2}")
                    nc.scalar.activation(out=mt_sb[:], in_=t_ps[:], func=AF.Copy)
                    mt.append(mt_sb)

                xsl0 = r["xt"][0][:, w * P: (w + 1) * P]
                xsl1 = r["xt"][1][:, w * P: (w + 1) * P]
                br = r["brow"]

                def dense(ps, parts, brow_slice):
                    for i, (lhsT, rhs) in enumerate(parts):
                        nc.tensor.matmul(out=ps, lhsT=lhsT, rhs=rhs,
                                         start=(i == 0), stop=False)
                    nc.tensor.matmul(out=ps, lhsT=ones_t[:], rhs=brow_slice,
                                     start=False, stop=True)
                    return ps

                pt = dpool.tile([P, D], f32, tag="p")
                qt = dpool.tile([P, D], f32, tag="q")
                zht = dpool.tile([P, D], f32, tag="zh")
                zat = dpool.tile([P, D], f32, tag="za")
                p_ps = dense(pt[:],
                             [(xsl0, r["wp_self"][0][:]), (xsl1, r["wp_self"][1][:]),
                              (mt[0][:], r["wp_rel"][0][:]), (mt[1][:], r["wp_rel"][1][:])],
                             br[:, 0:D])
                q_ps = dense(qt[:],
                             [(xsl0, r["wp_self"][0][:]), (xsl1, r["wp_self"][1][:]),
                              (mt[0][:], r["wq_rel"][0][:]), (mt[1][:], r["wq_rel"][1][:])],
                             br[:, 0:D])
                zh_ps = dense(zht[:],
                              [(xsl0, r["wf_self"][0][:]), (xsl1, r["wf_self"][1][:])],
                              br[:, D:2 * D])
                za_ps = dense(zat[:],
                              [(mt[0][:], r["wf_rel"][0][:]), (mt[1][:], r["wf_rel"][1][:])],
                              br[:, 2 * D:3 * D])

                th = sbpool.tile([P, D], f32, tag="th")
                nc.scalar.activation(out=th[:], in_=zh_ps, func=AF.Tanh)
                ta = sbpool.tile([P, D], f32, tag="ta")
                nc.scalar.activation(out=ta[:], in_=za_ps, func=AF.Tanh)

                v = sbpool.tile([P, D], f32, tag="v")
                nc.vector.tensor_tensor(out=v[:], in0=th[:], in1=ta[:],
                                        op=OP.subtract)
                vw = sbpool.tile([P, D], f32, tag="vw")
                nc.vector.tensor_tensor(out=vw[:], in0=v[:], in1=wrep_t[:],
                                        op=OP.mult)
                dsc = sbpool.tile([P, 1], f32, tag="dsc")
                nc.vector.tensor_reduce(out=dsc[:], in_=vw[:],
                                        axis=mybir.AxisListType.X, op=OP.add)

                t_sc = sbpool.tile([P, 1], f32, tag="tsc")
                nc.scalar.activation(out=t_sc[:], in_=dsc[:], func=AF.Tanh)

                wq = sbpool.tile([P, D], f32, tag="wq")
                nc.vector.tensor_scalar(out=wq[:], in0=q_ps,
                                        scalar1=t_sc[:, 0:1], scalar2=None,
                                        op0=OP.mult)
                outt = sbpool.tile([P, D], bf16, tag="outt")
                nc.vector.tensor_tensor(out=outt[:], in0=wq[:], in1=p_ps,
                                        op=OP.add)
                nc.sync.dma_start(r["out"][w * P: (w + 1) * P, :], outt[:])

            for _s in range(scale):
                for w in range(nwin):
                    for r in rels:
                        emit_window(w, r)

    nc.compile()
    return nc


# ---------------------------------------------------------------- driver
_PROG_CACHE = {}


def _get_program(key):
    if key not in _PROG_CACHE:
        _PROG_CACHE[key] = build_program(*key)
    return _PROG_CACHE[key]


def _make_in_maps(pr):
    shared = dict(
        xa8=pr["xa8"], xp8=pr["xp8"],
        iota=pr["iota"], ident=pr["ident"], ones=pr["ones"], wrep=pr["wrep"],
        brows_wr=pr["brows_wr"], brows_wn=pr["brows_wn"],
    )
    for tag in ("wr", "wn"):
        for nm in ("wp_self", "wp_rel", "wq_rel", "wf_self", "wf_rel"):
            shared[f"{nm}_{tag}"] = pr[f"{nm}_{tag}"]
    idx_wr, colf_wr, recip_wr, _, _ = pr["wr"]
    idx_wn, colf_wn, recip_wn, _, _ = pr["wn"]
    in_maps = []
    for c in range(NCORES):
        w0, w1 = c * NWIN, (c + 1) * NWIN
        m = dict(shared)
        m["xta"] = pr["xta"][c]
        m["xtp"] = pr["xtp"][c]
        m["idx_wr"] = np.ascontiguousarray(
            np.tile(idx_wr[:, w0:w1].reshape(16, -1), (8, 1)))
        m["idx_wn"] = np.ascontiguousarray(
            np.tile(idx_wn[:, w0:w1].reshape(16, -1), (8, 1)))
        m["colf_wr"] = np.ascontiguousarray(colf_wr[:, w0:w1].reshape(P, -1)).astype(BF16)
        m["colf_wn"] = np.ascontiguousarray(colf_wn[:, w0:w1].reshape(P, -1)).astype(BF16)
        m["recip_wr"] = np.ascontiguousarray(recip_wr[:, w0:w1])
        m["recip_wn"] = np.ascontiguousarray(recip_wn[:, w0:w1])
        in_maps.append(m)
    return in_maps


def run(trace=False, tmpdir=None, **inputs):
    pr = _host_prep(inputs)
    _, _, _, c_lo_wr, c_hi_wr = pr["wr"]
    _, _, _, c_lo_wn, c_hi_wn = pr["wn"]
    nc = _get_program((NWIN, c_lo_wr, c_hi_wr, c_lo_wn, c_hi_wn, 1, NQ, USE_FP8))
    in_maps = _make_in_maps(pr)
    res = run_bass_kernel_spmd(nc, in_maps, list(range(NCORES)),
                               trace=trace, tmpdir=tmpdir)
    oa = np.empty((N, D), dtype=F32)
    op = np.empty((N, D), dtype=F32)
    for c in range(NCORES):
        r0, r1 = c * NPAD, min(N, (c + 1) * NPAD)
        oa[r0:r1] = res.results[c]["oa"][: r1 - r0].astype(F32)
        op[r0:r1] = res.results[c]["op"][: r1 - r0].astype(F32)
    return (oa, op), res


def kernel(**inputs):
    (oa, op), _ = run(trace=False, **inputs)
    return (oa, op)
